# revision 4
# baseline (speedup 1.0000x reference)
"""Tensor-parallel MHSA (RoPE + causal attention) for 8 TRN2 NeuronCores, v2.

Sharding: 8-way tensor-parallel over heads (16 heads -> 2 per core), both
batches on every core.  Each core computes q/k/v projections for its 2 heads
(column-parallel), RoPE, causal attention, and a row-parallel slice of the
output projection, producing a full-shape partial y^T in bf16; the host sums
the 8 partials and adds bo + Wo^T bv (the v-bias folds out of attention since
softmax rows sum to 1).

Key structure:
- all matmuls bf16 (same PE rate as fp32r, half the DMA/SBUF of fp32)
- few, large DMAs (HWDGE issue overhead is ~625ns each): weights land in one
  pre-transposed [128, KT, COLS] transfer each, x in 1MB k-pair tiles,
  y in [128, 4eb, 512] groups, constants packed/memset
- QKV runs as three k-outer phases (q, k, v) over 8 single-bank PSUM
  accumulation chains, so each arriving x k-pair immediately feeds 32
  matmuls (hides the DMA ramp) and each stationary weight is reused 8x
  (amortizes LDWEIGHTS)
- each PSUM bank holds exactly ONE accumulation group (zero-region rule):
  both heads' chains in a bank share one start/stop pair
- q/k biases applied during PSUM evacuation (ACT Identity+bias / DVE
  tensor_scalar_add); v bias folds into the host-side output bias
- RoPE on half-rows with bf16 2x DVE tensor_tensor (sign folded into table)
- causal mask added by a tiny PE matmul (maskT^T @ I) inside the scores
  accumulation group
- attention: S^T layout, lookahead-2 block pipeline, exp on ACT -> bf16;
  out-proj for chunk lc-1 emitted inside chunk lc's first head so the PE
  never waits on the softmax-normalize chain
"""
import sys
sys.path.insert(0, "/opt/trn_rl_repo")
import numpy as np

B, L, E = 2, 2048, 2048
HEADS = 16
HD = 128
BASE = 10000.0
NCORES = 8
HPC = HEADS // NCORES      # heads per core = 2
COLS = HPC * HD            # 256 columns of Wq/Wk/Wv per core
KT = E // 128              # 16 k-tiles
LC = L // 512              # 4 l-chunks (attention / out-proj)
SC = L // 256              # 8 sub-chunks (qkv projection)
NEG = -1.0e9


def _build_program():
    import concourse.bass as bass
    import concourse.mybir as mybir
    import concourse.tile as tile
    from concourse import bacc

    F32 = mybir.dt.float32
    F32R = mybir.dt.float32r
    BF16 = mybir.dt.bfloat16
    Exp = mybir.ActivationFunctionType.Exp

    nc = bacc.Bacc()
    x_d = nc.declare_dram_parameter("xkt", [B, KT, 128, L], BF16, isOutput=False)
    wq_d = nc.declare_dram_parameter("wq", [128, KT, COLS], BF16, isOutput=False)
    wk_d = nc.declare_dram_parameter("wk", [128, KT, COLS], BF16, isOutput=False)
    wv_d = nc.declare_dram_parameter("wv", [128, KT, COLS], BF16, isOutput=False)
    wo_d = nc.declare_dram_parameter("wo", [128, HPC, E], BF16, isOutput=False)
    bias_d = nc.declare_dram_parameter("bias2", [128, 4], F32, isOutput=False)
    tabs_d = nc.declare_dram_parameter("tabs", [128, 2, L], BF16, isOutput=False)
    mi_d = nc.declare_dram_parameter("mi", [128, 256], BF16, isOutput=False)
    y_d = nc.declare_dram_parameter("yT", [B, E, L], BF16, isOutput=True)

    with nc.allow_low_precision(reason="bf16 matmuls"), \
         tile.TileContext(nc) as tc:
        with (
            tc.tile_pool(name="fixed", bufs=1) as fixed,
            tc.tile_pool(name="xs", bufs=1) as xs,
            tc.tile_pool(name="qkv", bufs=1) as qkvp,
            tc.tile_pool(name="rope", bufs=2) as rp,
            tc.tile_pool(name="bc", bufs=1) as bcp,
            tc.tile_pool(name="pt", bufs=4) as ptp,
            tc.tile_pool(name="ot", bufs=2) as otp,
            tc.tile_pool(name="yst", bufs=3) as yst,
            tc.tile_pool(name="small", bufs=2) as smallp,
        ):
            # DMA order = consumption order: wq, then x pairs (the q phase
            # tracks x arrival), then wk/wv, then late-use constants.
            wq_sb = fixed.tile([128, KT, COLS], BF16, name="wq", tag="wq")
            nc.sync.dma_start(out=wq_sb[:, 0:8, :], in_=wq_d[:, 0:8, :])
            nc.sync.dma_start(out=wq_sb[:, 8:KT, :], in_=wq_d[:, 8:KT, :])
            xk = [xs.tile([128, 2, L], BF16, name=f"xk{g}", tag=f"xk{g}")
                  for g in range(KT // 2)]
            for j in range(2):
                nc.sync.dma_start(
                    out=xk[0][:, j, :],
                    in_=x_d[0, j].rearrange("p l -> p l"))
            for g in range(1, KT // 2):
                nc.sync.dma_start(
                    out=xk[g],
                    in_=x_d[0, 2 * g:2 * g + 2].rearrange("k p l -> p k l"))
            wk_sb = fixed.tile([128, KT, COLS], BF16, name="wk", tag="wk")
            nc.sync.dma_start(out=wk_sb, in_=wk_d[:, :, :])
            wv_sb = fixed.tile([128, KT, COLS], BF16, name="wv", tag="wv")
            nc.sync.dma_start(out=wv_sb, in_=wv_d[:, :, :])
            bias_sb = fixed.tile([128, 4], F32, name="bias2", tag="bias2")
            nc.sync.dma_start(out=bias_sb, in_=bias_d[:, :])
            tabs_sb = fixed.tile([128, 2, L], BF16, name="tabs", tag="tabs")
            nc.sync.dma_start(out=tabs_sb, in_=tabs_d[:, :, :])
            mi_sb = fixed.tile([128, 256], BF16, name="mi", tag="mi")
            nc.sync.dma_start(out=mi_sb, in_=mi_d[:, :])
            wo_sb = fixed.tile([128, HPC, E], BF16, name="wo", tag="wo")
            nc.sync.dma_start(out=wo_sb, in_=wo_d[:, :, :])

            cos_sb = tabs_sb[:, 0, :]
            sin_sb = tabs_sb[:, 1, :]
            mask_sb = mi_sb[:, 0:128]
            id_sb = mi_sb[:, 128:256]
            ones_sb = fixed.tile([128, 1], BF16, name="onesb", tag="onesb")
            nc.vector.memset(ones_sb, 1.0)

            def xkt_ap(k):
                return xk[k // 2][:, k % 2, :]

            for b in range(B):

                qT = qkvp.tile([128, HPC, L], BF16, name="qT", tag="qT")
                kT = qkvp.tile([128, HPC, L], BF16, name="kT", tag="kT")
                qraw = qkvp.tile([128, HPC, L], BF16, name="qraw", tag="qraw")
                kraw = qkvp.tile([128, HPC, L], BF16, name="kraw", tag="kraw")
                qsw = qkvp.tile([128, HPC, L], BF16, name="qsw", tag="qsw")
                ksw = qkvp.tile([128, HPC, L], BF16, name="ksw", tag="ksw")
                vv = qkvp.tile([128, 16, COLS], BF16, name="vv", tag="vv")

                def rope_chunk(hc):
                    ch = slice(hc * 1024, hc * 1024 + 1024)
                    for src, ssw in ((qraw, qsw), (kraw, ksw)):
                        # half-swap staged by DMA (engines cannot cross base
                        # partitions between SBUF operands; DMA can)
                        nc.sync.dma_start(out=ssw[0:64, :, ch],
                                          in_=src[64:128, :, ch])
                        nc.sync.dma_start(out=ssw[64:128, :, ch],
                                          in_=src[0:64, :, ch])
                    for h in range(HPC):
                        for src, ssw, dst in ((qraw, qsw, qT), (kraw, ksw, kT)):
                            t1 = rp.tile([128, 1024], BF16, name="t1", tag="t1")
                            nc.vector.tensor_mul(t1, ssw[:, h, ch], sin_sb[:, ch])
                            t2 = rp.tile([128, 1024], BF16, name="t2", tag="t2")
                            nc.vector.tensor_mul(t2, src[:, h, ch], cos_sb[:, ch])
                            nc.vector.tensor_add(dst[:, h, ch], t1, t2)

                # ---------- QKV projection over 8 PSUM banks ----------
                # q phase is k-outer so each arriving x k-pair feeds all 8
                # chains (tracks the DMA ramp); k and v phases run sc-outer
                # (x already resident) so their evacuations spread across the
                # phase instead of bunching at its end.
                with tc.tile_pool(name=f"psq{b}", bufs=1, space="PSUM") as psq:
                    qps = [psq.tile([128, HPC, 256], F32,
                                    name=f"ps{sc}", tag=f"ps{sc}")
                           for sc in range(SC)]
                    for k in range(KT):
                        for h in range(HPC):
                            for sc in range(SC):
                                nc.tensor.matmul(
                                    qps[sc][:, h, :],
                                    lhsT=wq_sb[:, k, h * 128:(h + 1) * 128],
                                    rhs=xkt_ap(k)[:, sc * 256:(sc + 1) * 256],
                                    start=(k == 0 and h == 0),
                                    stop=(k == KT - 1 and h == HPC - 1))
                    for sc in range(SC):
                        sl = slice(sc * 256, (sc + 1) * 256)
                        nc.scalar.add(out=qraw[:, 0, sl], in_=qps[sc][:, 0, :],
                                      add=bias_sb[:, 0:1])
                        nc.vector.tensor_scalar_add(
                            out=qraw[:, 1, sl], in0=qps[sc][:, 1, :],
                            scalar1=bias_sb[:, 1:2])

                    for sc in range(SC):
                        kps = psq.tile([128, HPC, 256], F32,
                                       name=f"ps{sc}", tag=f"ps{sc}")
                        sl = slice(sc * 256, (sc + 1) * 256)
                        for k in range(KT):
                            for h in range(HPC):
                                nc.tensor.matmul(
                                    kps[:, h, :],
                                    lhsT=wk_sb[:, k, h * 128:(h + 1) * 128],
                                    rhs=xkt_ap(k)[:, sl],
                                    start=(k == 0 and h == 0),
                                    stop=(k == KT - 1 and h == HPC - 1))
                        nc.scalar.add(out=kraw[:, 0, sl], in_=kps[:, 0, :],
                                      add=bias_sb[:, 2:3])
                        nc.vector.tensor_scalar_add(
                            out=kraw[:, 1, sl], in0=kps[:, 1, :],
                            scalar1=bias_sb[:, 3:4])
                        if sc == 3:
                            rope_chunk(0)
                    rope_chunk(1)

                    # v phase: V in token-major (x slices stationary)
                    for sc in range(SC):
                        vps = psq.tile([128, 2, 256], F32,
                                       name=f"ps{sc}", tag=f"ps{sc}")
                        for k in range(KT):
                            for i in range(2):
                                nc.tensor.matmul(
                                    vps[:, i, :],
                                    lhsT=xkt_ap(k)[:, sc * 256 + i * 128:
                                                   sc * 256 + (i + 1) * 128],
                                    rhs=wv_sb[:, k, :],
                                    start=(k == 0 and i == 0),
                                    stop=(k == KT - 1 and i == 1))
                        nc.scalar.copy(out=vv[:, 2 * sc:2 * sc + 2, :],
                                       in_=vps)

                if b + 1 < B:
                    for g in range(KT // 2):
                        nc.sync.dma_start(
                            out=xk[g],
                            in_=x_d[b + 1, 2 * g:2 * g + 2]
                            .rearrange("k p l -> p k l"))

                # ---------- causal attention + interleaved out-proj ----------
                oTs = {}

                def out_proj_group(lc, ebg, gsz=4):
                    if True:
                        ys = yst.tile([128, 4, 512], BF16, name="ys", tag="ys")
                        for i in range(gsz):
                            eb = ebg * gsz + i
                            yp = psy.tile([128, 512], F32, name="yp", tag="yp")
                            for h in range(HPC):
                                nc.tensor.matmul(
                                    yp,
                                    lhsT=wo_sb[:, h, eb * 128:(eb + 1) * 128],
                                    rhs=oTs[(lc, h)],
                                    start=(h == 0), stop=(h == HPC - 1))
                            if lc == LC - 1 and i % 2 == 0:
                                nc.scalar.copy(out=ys[:, i, :], in_=yp)
                            else:
                                nc.vector.tensor_copy(ys[:, i, :], yp)
                        nc.sync.dma_start(
                            out=y_d[b, ebg * gsz * 128:(ebg + 1) * gsz * 128,
                                    lc * 512:(lc + 1) * 512]
                            .rearrange("(e p) l -> p e l", p=128),
                            in_=ys[:, 0:gsz, :])

                with (
                    tc.tile_pool(name=f"pss{b}", bufs=3, space="PSUM") as pss,
                    tc.tile_pool(name=f"psa{b}", bufs=2, space="PSUM") as psa,
                    tc.tile_pool(name=f"psr{b}", bufs=1, space="PSUM") as psr,
                    tc.tile_pool(name=f"psy{b}", bufs=2, space="PSUM") as psy,
                ):
                    for lc in range(LC):
                        for h in range(HPC):
                            av = psa.tile([128, 512], F32, name="av", tag="av")
                            rs = psr.tile([1, 512], F32, name="rs", tag="rs")
                            nmb = 4 * lc + 4
                            pend = []

                            def flush(av=av, rs=rs, nmb=nmb, h=h):
                                mb, pt, npr, c0 = pend.pop(0)
                                nc.tensor.matmul(
                                    av[:, c0:512],
                                    lhsT=vv[:, mb, h * 128:(h + 1) * 128],
                                    rhs=pt[:, 0:npr], start=(mb == 0),
                                    stop=(mb == nmb - 1))
                                nc.tensor.matmul(
                                    rs[0:1, c0:512], lhsT=ones_sb,
                                    rhs=pt[:, 0:npr], start=(mb == 0),
                                    stop=(mb == nmb - 1))

                            for mb in range(nmb):
                                l0 = max(lc * 512, mb * 128)
                                npr = lc * 512 + 512 - l0
                                c0 = l0 - lc * 512
                                diag = mb >= 4 * lc
                                st = pss.tile([128, 512], F32, name="st", tag="st")
                                nc.tensor.matmul(
                                    st[:, 0:npr],
                                    lhsT=kT[:, h, mb * 128:(mb + 1) * 128],
                                    rhs=qT[:, h, l0:l0 + npr],
                                    start=True, stop=not diag)
                                if diag:
                                    nc.tensor.matmul(
                                        st[:, 0:128], lhsT=mask_sb, rhs=id_sb,
                                        start=False, stop=True)
                                pt = ptp.tile([128, 512], BF16, name="pt", tag="pt")
                                nc.scalar.activation(
                                    out=pt[:, 0:npr], in_=st[:, 0:npr], func=Exp)
                                pend.append((mb, pt, npr, c0))
                                if len(pend) >= 3:
                                    flush()
                                # slot the previous chunk's out-proj groups
                                # into the last 4 block iterations so the PE
                                # never waits on the normalize chain and the
                                # evac copies spread across the DVE queue
                                if h == 0 and lc > 0 and mb >= nmb - 4:
                                    out_proj_group(lc - 1, mb - (nmb - 4))
                            while pend:
                                flush()

                            rec = smallp.tile([1, 512], F32, name="rec", tag="rec")
                            nc.vector.reciprocal(out=rec, in_=rs)
                            bcs = bcp.tile([128, 512], F32, name="bcs", tag="bcs")
                            nc.gpsimd.partition_broadcast(bcs, rec, channels=128)
                            oTt = otp.tile([128, 512], BF16,
                                           name=f"oT{lc}_{h}", tag=f"oT{lc}_{h}")
                            nc.vector.tensor_mul(oTt, av, bcs)
                            oTs[(lc, h)] = oTt
                    for ebg in range(3):
                        out_proj_group(LC - 1, ebg)
                    out_proj_group(LC - 1, 6, gsz=2)
                    out_proj_group(LC - 1, 7, gsz=2)
    nc.compile()
    return nc


_NC_CACHE = None


def build_in_maps(x, Wq, bq, Wk, bk, Wv, bv, Wo, bo):
    import ml_dtypes

    BF = ml_dtypes.bfloat16
    x = np.asarray(x, np.float32)
    scale = HD ** (-0.5)

    inv = 1.0 / (BASE ** (np.arange(0, HD, 2, dtype=np.float32) / HD))
    fr = np.outer(inv, np.arange(L, dtype=np.float32))          # [64, L]
    cosf = np.cos(fr).astype(np.float32)
    sinf = np.sin(fr).astype(np.float32)
    cos2 = np.concatenate([cosf, cosf], 0)                      # [128, L]
    sinpm = np.concatenate([-sinf, sinf], 0)                    # [128, L]
    tabs = np.ascontiguousarray(
        np.stack([cos2, sinpm], 1)).astype(BF)                  # [128, 2, L]
    mask = np.where(np.arange(128)[:, None] <= np.arange(128)[None, :],
                    0.0, NEG).astype(np.float32)
    mi = np.ascontiguousarray(
        np.concatenate([mask.T, np.eye(128, dtype=np.float32)], 1)).astype(BF)

    xT = np.transpose(x, (0, 2, 1))                             # [B, E, L]
    xkt = np.ascontiguousarray(
        xT.reshape(B, KT, 128, L)).astype(BF)                   # [B, KT, 128, L]

    Wq = np.asarray(Wq, np.float32)
    Wk = np.asarray(Wk, np.float32)
    Wv = np.asarray(Wv, np.float32)
    Wo = np.asarray(Wo, np.float32)
    bq = np.asarray(bq, np.float32)
    bk = np.asarray(bk, np.float32)
    bv = np.asarray(bv, np.float32)
    bo = np.asarray(bo, np.float32)

    in_maps = []
    for c in range(NCORES):
        cols = slice(c * COLS, (c + 1) * COLS)
        # weights pre-transposed to [128 partition, KT, COLS] so each lands
        # in one long-run DMA
        wq_c = np.ascontiguousarray(
            (Wq[:, cols] * scale).reshape(KT, 128, COLS)
            .transpose(1, 0, 2)).astype(BF)
        wk_c = np.ascontiguousarray(
            Wk[:, cols].reshape(KT, 128, COLS).transpose(1, 0, 2)).astype(BF)
        wv_c = np.ascontiguousarray(
            Wv[:, cols].reshape(KT, 128, COLS).transpose(1, 0, 2)).astype(BF)
        wo_c = np.ascontiguousarray(
            Wo[cols, :].reshape(HPC, 128, E).transpose(1, 0, 2)).astype(BF)
        # biases as [128 partition, (bq h0, bq h1, bk h0, bk h1, then the
        # same four half-swapped for the qsw/ksw evacuations)] f32
        bqh = (bq[cols] * scale).reshape(HPC, 128)
        bkh = bk[cols].reshape(HPC, 128)
        bias2 = np.ascontiguousarray(np.stack(
            [bqh[0], bqh[1], bkh[0], bkh[1]], 1)).astype(np.float32)
        in_maps.append({
            "xkt": xkt,
            "wq": wq_c, "wk": wk_c, "wv": wv_c, "wo": wo_c,
            "bias2": bias2, "tabs": tabs, "mi": mi,
        })
    return in_maps


def kernel(x, Wq, bq, Wk, bk, Wv, bv, Wo, bo):
    global _NC_CACHE
    from concourse.bass_utils import run_bass_kernel_spmd

    in_maps = build_in_maps(x, Wq, bq, Wk, bk, Wv, bv, Wo, bo)
    Wo = np.asarray(Wo, np.float32)
    bv = np.asarray(bv, np.float32)
    bo = np.asarray(bo, np.float32)

    if _NC_CACHE is None:
        _NC_CACHE = _build_program()
    res = run_bass_kernel_spmd(_NC_CACHE, in_maps, list(range(NCORES)))
    acc = np.zeros((B, E, L), np.float64)
    for c in range(NCORES):
        acc += res.results[c]["yT"].astype(np.float32)
    # v-bias folds out of attention (softmax rows sum to 1): out@Wo picks up
    # the constant bv@Wo term, added here in full precision along with bo.
    bias = bo + bv @ Wo
    y = (np.transpose(acc, (0, 2, 1)) + bias).astype(np.float32)
    return y
